# revision 1
# baseline (speedup 1.0000x reference)
"""AttnBlock (GroupNorm + single-head self-attention + residual) on 8 Trainium2 cores.

Sharding: core i handles batch b = i//2 and query-half h = i%2 (2048 of 4096
pixels). Each core computes full-batch groupnorm stats + K/V^T, its half of Q,
attention over all 4096 keys for its 2048 queries, and the output projection.
Host does the final bias + residual add and gathers.

v3 pipeline:
  - Softmax bias algebra: the K-projection bias adds a per-query constant to
    the scores, which softmax cancels -> K carries NO bias (pure scale evict).
    Softmax weights sum to 1, so the V-projection bias contributes a constant
    per channel: out += Wo@(Wv@b_gn + bv) + bo, all folded into the host-side
    bias add (host recomputes GN stats from f32 x, closer to the reference
    than the device fp8 path).
  - Engine rebalance: evictions spread Pool/DVE/ACT; GN stats DVE sums + ACT
    squares; 6/16 exp tiles per qt on Pool via the integer fp8-exp trick.
  - softmax 1/Z transposed on-chip with PE transposes (no DRAM bounce); the
    transpose+copy is deferred into the next qt's stream to hide latency.
All matmuls fp8 DoubleRow (fp32 PSUM accumulation).
"""

import numpy as np
import ml_dtypes

C = 512
HW = 4096
HWQ = 2048
CCH = 4          # channel chunks of 128
KT = 32          # key tiles of 128
QT = 4           # query tiles of 512
NCORES = 8
GROUPS = 32
GS = 16          # channels per group
EPS = 1e-5
SCALE = 1.0 / float(np.sqrt(C))
SCALE_H = float(SCALE ** 0.5)
OSC = 1.0 / 32.0          # o scaled into fp8 range; undone via the 1/Z multiply
EXP_A = float(8.0 / np.log(2.0))   # integer-exp trick: P = trunc(s*EXP_A + EXP_B)
EXP_B = 56.0
DVE_EXP_PAIRS = frozenset(range(1, 16, 2))  # odd pairs on DVE int-exp, even on ACT

_cache = {}


def _emit_body(nc, tc, bassmod, mybir, ctx, T):
    """Emit one full forward pass. T is the dict of dram tensor handles."""
    bass = bassmod
    f32 = mybir.dt.float32
    f8 = mybir.dt.float8e4
    f16 = mybir.dt.float16
    u8 = mybir.dt.uint8
    AF = mybir.ActivationFunctionType
    ALU = mybir.AluOpType
    DR = mybir.MatmulPerfMode.DoubleRow

    # ---------------- pools ----------------
    consts = ctx.enter_context(tc.tile_pool(name="consts", bufs=1))
    xb = ctx.enter_context(tc.tile_pool(name="xb", bufs=1))
    ps_s = ctx.enter_context(tc.tile_pool(name="ps_s", bufs=3, space="PSUM"))
    ps_o = ctx.enter_context(tc.tile_pool(name="ps_o", bufs=4, space="PSUM"))
    ps_z = ctx.enter_context(tc.tile_pool(name="ps_z", bufs=1, space="PSUM"))
    kpool = ctx.enter_context(tc.tile_pool(name="kpool", bufs=1))
    qpool = ctx.enter_context(tc.tile_pool(name="qpool", bufs=1))
    vpool = ctx.enter_context(tc.tile_pool(name="vpool", bufs=KT // 2))
    opool = ctx.enter_context(tc.tile_pool(name="opool", bufs=1))
    epool = ctx.enter_context(tc.tile_pool(name="epool", bufs=6))
    outp = ctx.enter_context(tc.tile_pool(name="outp", bufs=3))
    rzp = ctx.enter_context(tc.tile_pool(name="rzp", bufs=2))
    spool = ctx.enter_context(tc.tile_pool(name="spool", bufs=1))
    tmpp = ctx.enter_context(tc.tile_pool(name="tmpp", bufs=2))
    wpool = ctx.enter_context(tc.tile_pool(name="wpool", bufs=2))

    # ---------------- input DMAs ----------------
    cc_sb = consts.tile([128, 16], f32, tag="colconsts", name="colconsts")
    nc.sync.dma_start(out=cc_sb, in_=T["colc"][:, :])
    gadj_sb = consts.tile([128, 128], f32, tag="gadj", name="gadj")
    nc.sync.dma_start(out=gadj_sb, in_=T["gadj"][:, :])
    gnw_c = [cc_sb[:, 4 * ci + 0:4 * ci + 1] for ci in range(CCH)]
    gnb_c = [cc_sb[:, 4 * ci + 1:4 * ci + 2] for ci in range(CCH)]
    bq_c = [cc_sb[:, 4 * ci + 2:4 * ci + 3] for ci in range(CCH)]
    ones2_sb = consts.tile([128, 2, 16], f8, tag="ones2", name="ones2")
    nc.vector.memset(ones2_sb, 1.0)
    onesf_sb = consts.tile([1, 128], f32, tag="onesf", name="onesf")
    nc.vector.memset(onesf_sb, 1.0)
    eps_sb = consts.tile([128, 1], f32, tag="eps", name="eps")
    nc.vector.memset(eps_sb, EPS)

    # x as fp8: one tile [128, ci, pixel], DMA'd per-ci so stats chase the DMA
    xkv_v = xb.tile([128, CCH, HW], f8, tag="xkv", name="xkv")
    for ci in range(CCH):
        nc.sync.dma_start(out=xkv_v[:, ci, :],
                          in_=bass.AP(T["xkv"], ci * 128 * HW, [[HW, 128], [1, HW]]))
    # fp8 weights for q/k/v: [128, nm, ci, co]; fp8 weights for o
    wall = consts.tile([128, 3 * CCH * 512], f8, tag="wall", name="wall")
    nc.sync.dma_start(out=wall, in_=T["wall"][:, :])
    wv_q = {nm: wall[:, i * 2048:(i + 1) * 2048].rearrange("p (c w) -> p c w", c=CCH)
            for i, nm in enumerate(("wkt", "wvt", "wqt"))}
    wot = consts.tile([128, CCH, 512], f8, tag="wot", name="wot")
    nc.sync.dma_start(out=wot, in_=T["wotp"][:, :].rearrange("p (c w) -> p c w", c=CCH))
    xq_v = xb.tile([128, CCH, HWQ], f8, tag="xq", name="xq")
    nc.sync.dma_start(out=xq_v,
                      in_=bass.AP(T["xq"], 0, [[HWQ, 128], [128 * HWQ, CCH], [1, HWQ]]))

    # PE warm-up: HAM needs ~3.4us of activity to unthrottle, and re-throttles
    # after ~3.4us idle. Gate each dummy-matmul round on a stats artifact so
    # rounds fire as the stats pipeline progresses (PE idle stays < ~3us).
    ps_w = ps_z.tile([1, 512], f32, tag="z", name="z")
    warm = wpool.tile([128, 2, 512], f8, tag="warm", name="warm")
    nc.gpsimd.memset(warm, 0.25)
    _warm_state = {"first": True}

    def warm_round(gate=None, n=9, last=False):
        if gate is not None:
            # tiny Pool SBUF-copy stamps the warm tile, so this round's
            # matmuls (which read it) wait for the stats artifact
            nc.gpsimd.tensor_copy(warm[:, 0, 0:1], gate)
        for i in range(n):
            nc.tensor.matmul(out=ps_w, lhsT=ones2_sb[:, :, 0:1],
                             rhs=warm, perf_mode=DR, start=_warm_state["first"],
                             stop=(last and i == n - 1), skip_group_check=True)
            _warm_state["first"] = False

    # early rounds paced by chained Pool memsets (~1.7us apart) until the
    # first stats artifacts exist to gate on
    warm_round()
    for _ in range(4):
        nc.gpsimd.memset(warm, 0.25)
        nc.gpsimd.memset(warm, 0.25)
        warm_round()

    # ---------------- groupnorm stats ----------------
    # per ci: sum on DVE, sum-of-squares on ACT.
    a_pc = []
    b8 = spool.tile([128, CCH], f8, tag="b8", name="b8")
    st_t = [spool.tile([128, 2], f32, tag=f"st{ci}", name=f"st{ci}") for ci in range(CCH)]
    for ci in range(CCH):
        nc.vector.reduce_sum(out=st_t[ci][:, 0:1], in_=xkv_v[:, ci, :],
                             axis=mybir.AxisListType.X)
        scr = tmpp.tile([128, HW], f8, tag="scr", name="scr")
        nc.scalar.activation(out=scr, in_=xkv_v[:, ci, :], func=AF.Square,
                             accum_out=st_t[ci][:, 1:2])
        warm_round(gate=scr[:, 0:1])

    for ci in range(CCH):
        ps_g = ps_s.tile([128, 2], f32, tag="ps", name="ps")
        nc.tensor.matmul(out=ps_g, lhsT=gadj_sb, rhs=st_t[ci], start=True, stop=True)
        gs = spool.tile([128, 2], f32, tag=f"gs{ci}", name=f"gs{ci}")
        nc.scalar.mul(out=gs, in_=ps_g, mul=1.0 / (GS * HW))   # [mu | E[x^2]]
        var = spool.tile([128, 1], f32, tag=f"var{ci}", name=f"var{ci}")
        nc.vector.tensor_mul(var, gs[:, 0:1], gs[:, 0:1])
        nc.vector.tensor_sub(var, gs[:, 1:2], var)
        sd = spool.tile([128, 1], f32, tag=f"sd{ci}", name=f"sd{ci}")
        nc.scalar.activation(out=sd, in_=var, func=AF.Sqrt, bias=eps_sb, scale=1.0)
        rstd = spool.tile([128, 1], f32, tag=f"rstd{ci}", name=f"rstd{ci}")
        nc.vector.reciprocal(out=rstd, in_=sd)
        a = spool.tile([128, 1], f32, tag=f"apc{ci}", name=f"apc{ci}")
        nc.vector.tensor_mul(a, rstd, gnw_c[ci])
        b = spool.tile([128, 1], f32, tag=f"bpc{ci}", name=f"bpc{ci}")
        nc.vector.tensor_mul(b, gs[:, 0:1], a)
        nc.vector.tensor_sub(b, gnb_c[ci], b)
        nc.vector.tensor_copy(b8[:, ci:ci + 1], b)
        a_pc.append(a)
        warm_round(gate=b8[:, ci:ci + 1], last=(ci == CCH - 1))

    # effective Q bias: bqeff = SCALE_H*(Wq @ b + bq)  (K/V biases fold out)
    bqeff = []
    for co in range(CCH):
        psq = ps_s.tile([128, 1], f32, tag="ps", name="ps")
        for ci in range(CCH):
            nc.tensor.matmul(out=psq, lhsT=wv_q["wqt"][:, ci, co * 128:(co + 1) * 128],
                             rhs=b8[:, ci:ci + 1], start=(ci == 0), stop=(ci == CCH - 1))
        bq = spool.tile([128, 1], f32, tag=f"bqe{co}", name=f"bqe{co}")
        nc.scalar.activation(out=bq, in_=psq, func=AF.Identity, bias=bq_c[co], scale=SCALE_H)
        bqeff.append(bq)

    # scale q/k/v weight rows by a (in place, after the bq matvecs read wqt)
    # ci 0/1 on Pool (frees earliest), ci 2 on DVE, ci 3 on ACT
    for ci in range(CCH):
        for nm in ("wkt", "wvt", "wqt"):
            w_sl = wv_q[nm][:, ci, :]
            nc.gpsimd.tensor_scalar_mul(w_sl, w_sl, a_pc[ci])

    # ---------------- projections (fp8 DoubleRow) ----------------
    ksb = kpool.tile([128, CCH, HW], f8, tag="ksb", name="ksb")
    qsb = qpool.tile([128, CCH, HWQ], f8, tag="qsb", name="qsb")
    vsb = [vpool.tile([128, 2, C], f8, tag="vt", name="vt") for _ in range(KT // 2)]

    def emit_vtile(kt, eng):
        ps = ps_s.tile([128, 512], f32, tag="ps", name="ps")
        for j in range(2):
            nc.tensor.matmul(out=ps,
                             lhsT=xkv_v[:, 2 * j:2 * j + 2, kt * 128:(kt + 1) * 128],
                             rhs=wv_q["wvt"][:, 2 * j:2 * j + 2, :],
                             perf_mode=DR, start=(j == 0), stop=(j == 1))
        dst = vsb[kt // 2][:, kt % 2, :]
        if eng == "dve":
            nc.vector.tensor_scalar_mul(dst, ps, 1.0)
        else:
            nc.scalar.activation(out=dst, in_=ps, func=AF.Identity, scale=1.0)

    def emit_ktile(pt, co, eng):
        ps = ps_s.tile([128, 512], f32, tag="ps", name="ps")
        for j in range(2):
            nc.tensor.matmul(out=ps,
                             lhsT=wv_q["wkt"][:, 2 * j:2 * j + 2, co * 128:(co + 1) * 128],
                             rhs=xkv_v[:, 2 * j:2 * j + 2, pt * 512:(pt + 1) * 512],
                             perf_mode=DR, start=(j == 0), stop=(j == 1))
        dst = ksb[:, co, pt * 512:(pt + 1) * 512]
        if eng == "dve":
            nc.vector.tensor_scalar_mul(dst, ps, SCALE_H)
        else:
            nc.scalar.activation(out=dst, in_=ps, func=AF.Identity, scale=SCALE_H)

    def emit_qtile(pt, co, eng):
        ps = ps_s.tile([128, 512], f32, tag="ps", name="ps")
        for j in range(2):
            nc.tensor.matmul(out=ps,
                             lhsT=wv_q["wqt"][:, 2 * j:2 * j + 2, co * 128:(co + 1) * 128],
                             rhs=xq_v[:, 2 * j:2 * j + 2, pt * 512:(pt + 1) * 512],
                             perf_mode=DR, start=(j == 0), stop=(j == 1))
        dst = qsb[:, co, pt * 512:(pt + 1) * 512]
        if eng == "dve":
            nc.vector.tensor_scalar(out=dst, in0=ps, scalar1=SCALE_H, scalar2=bqeff[co],
                                    op0=ALU.mult, op1=ALU.add)
        else:
            nc.scalar.activation(out=dst, in_=ps, func=AF.Identity, bias=bqeff[co],
                                 scale=SCALE_H)

    # V first (evicts don't wait on any bias), then K in scores-consumption
    # order, then Q pt0.
    KE = ["act", "dve", "act", "dve"]
    for g in range(8):
        emit_vtile(4 * g + 0, "dve")
        emit_vtile(4 * g + 1, "act")
        emit_vtile(4 * g + 2, "dve")
        emit_vtile(4 * g + 3, "act")
    for pt in range(8):
        for co in range(CCH):
            emit_ktile(pt, co, KE[co])
    for co in range(CCH):
        emit_qtile(0, co, ("act", "dve", "act", "dve")[co])

    # ---------------- attention (+ deferred per-tile output projection) ----------------
    def emit_oproj(qt, o_qt, rzt, qcs=(0, 1, 2, 3)):
        for qc in qcs:
            ps = ps_s.tile([128, 512], f32, tag="ps", name="ps")
            for j in range(2):
                nc.tensor.matmul(out=ps, lhsT=o_qt[:, 2 * j:2 * j + 2, qc * 128:(qc + 1) * 128],
                                 rhs=wot[:, 2 * j:2 * j + 2, :], perf_mode=DR,
                                 start=(j == 0), stop=(j == 1))
            ot = outp.tile([128, 512], f16, tag="ot", name="ot")
            nc.vector.tensor_scalar_mul(ot, ps, rzt[:, qc:qc + 1])
            row0 = qt * 512 + qc * 128
            nc.sync.dma_start(out=T["outt"][row0:row0 + 128, :], in_=ot)

    def emit_rz_tail(rz_row):
        # 4 PE transposes + copy: [1,512] 1/Z row -> [128,4] per-partition
        ps_rz = ps_s.tile([128, 4], f32, tag="ps", name="ps")
        for qc in range(4):
            nc.tensor.matmul(out=ps_rz[:, qc:qc + 1],
                             lhsT=rz_row[:, qc * 128:(qc + 1) * 128],
                             rhs=onesf_sb[:, 0:1],
                             is_transpose=True, start=True, stop=True)
        rzt = rzp.tile([128, 4], f32, tag="rzt", name="rzt")
        nc.scalar.activation(out=rzt, in_=ps_rz, func=AF.Identity, scale=1.0 / OSC)
        return rzt

    pending = None      # (qt, o_qt, rz_row) awaiting transpose + o-proj
    for qt in range(QT):
        ps_ot = [ps_o.tile([128, 512], f32, tag="pso", name="pso") for _ in range(CCH)]
        ps_zt = ps_z.tile([1, 512], f32, tag="z", name="z")
        pend = []     # consume exps two pairs late to hide exp latency
        for p in range(KT // 2):
            e_pair = epool.tile([128, 2, 512], f8, tag="e", name="e")
            for r in range(2):
                kt = 2 * p + r
                ps_st = ps_s.tile([128, 512], f32, tag="ps", name="ps")
                for j in range(2):
                    nc.tensor.matmul(out=ps_st,
                                     lhsT=ksb[:, 2 * j:2 * j + 2, kt * 128:(kt + 1) * 128],
                                     rhs=qsb[:, 2 * j:2 * j + 2, qt * 512:(qt + 1) * 512],
                                     perf_mode=DR, start=(j == 0), stop=(j == 1))
                if p in DVE_EXP_PAIRS:
                    nc.vector.tensor_scalar(out=e_pair[:, r, :].bitcast(u8), in0=ps_st,
                                            scalar1=EXP_A, scalar2=EXP_B,
                                            op0=ALU.mult, op1=ALU.add)
                else:
                    nc.scalar.activation(out=e_pair[:, r, :], in_=ps_st, func=AF.Exp)
            pend.append((p, e_pair))
            if len(pend) > 2:
                ppair, pe = pend.pop(0)
                nc.tensor.matmul(out=ps_zt, lhsT=ones2_sb[:, :, 0:1], rhs=pe, perf_mode=DR,
                                 start=(ppair == 0), stop=False, skip_group_check=True)
                for cc in range(CCH):
                    nc.tensor.matmul(out=ps_ot[cc],
                                     lhsT=vsb[ppair][:, :, cc * 128:(cc + 1) * 128],
                                     rhs=pe, perf_mode=DR, start=(ppair == 0),
                                     stop=False, skip_group_check=True)
            if qt == 0 and p in (4, 8, 12):
                pq = p // 4
                for co in range(CCH):
                    emit_qtile(pq, co, ("act", "dve", "act", "dve")[co])
            if qt > 0 and p == 3 and pending is not None:
                pqt, po_qt, prz_row = pending
                przt = emit_rz_tail(prz_row)
                pending = (pqt, po_qt, przt)
            if qt > 0 and p == 4 and pending is not None:
                emit_oproj(*pending, qcs=(0, 1))
            if qt > 0 and p == 10 and pending is not None:
                emit_oproj(*pending, qcs=(2, 3))
                pending = None
        while pend:
            ppair, pe = pend.pop(0)
            last = not pend
            nc.tensor.matmul(out=ps_zt, lhsT=ones2_sb[:, :, 0:1], rhs=pe, perf_mode=DR,
                             start=False, stop=last, skip_group_check=True)
            for cc in range(CCH):
                nc.tensor.matmul(out=ps_ot[cc],
                                 lhsT=vsb[ppair][:, :, cc * 128:(cc + 1) * 128],
                                 rhs=pe, perf_mode=DR, start=False, stop=last,
                                 skip_group_check=True)
        rz_row = rzp.tile([1, 512], f32, tag="rzrow", name="rzrow")
        nc.vector.reciprocal(out=rz_row, in_=ps_zt)
        o_qt = opool.tile([128, CCH, 512], f8, tag=f"o{qt}", name=f"o{qt}")
        if qt < QT - 1:
            for cc in range(CCH):
                if cc % 2 == 0:
                    nc.vector.tensor_scalar_mul(o_qt[:, cc, :], ps_ot[cc], OSC)
                else:
                    nc.scalar.activation(out=o_qt[:, cc, :], in_=ps_ot[cc],
                                         func=AF.Identity, scale=OSC)
        pending = (qt, o_qt, rz_row)
    # final tile: per-qc slice copies interleaved with its output projection
    qt, o_qt, rz_row = pending
    rzt = emit_rz_tail(rz_row)
    for qc in range(4):
        for cc in range(CCH):
            if cc % 2 == 0:
                nc.vector.tensor_scalar_mul(o_qt[:, cc, qc * 128:(qc + 1) * 128],
                                            ps_ot[cc][:, qc * 128:(qc + 1) * 128], OSC)
            else:
                nc.scalar.activation(out=o_qt[:, cc, qc * 128:(qc + 1) * 128],
                                     in_=ps_ot[cc][:, qc * 128:(qc + 1) * 128],
                                     func=AF.Identity, scale=OSC)
        ps = ps_s.tile([128, 512], f32, tag="ps", name="ps")
        for j in range(2):
            nc.tensor.matmul(out=ps, lhsT=o_qt[:, 2 * j:2 * j + 2, qc * 128:(qc + 1) * 128],
                             rhs=wot[:, 2 * j:2 * j + 2, :], perf_mode=DR,
                             start=(j == 0), stop=(j == 1))
        ot = outp.tile([128, 512], f16, tag="ot", name="ot")
        nc.vector.tensor_scalar_mul(ot, ps, rzt[:, qc:qc + 1])
        row0 = qt * 512 + qc * 128
        nc.sync.dma_start(out=T["outt"][row0:row0 + 128, :], in_=ot)


def build_program(repeat=1):
    import concourse.bacc as bacc
    import concourse.tile as tile
    from concourse import mybir
    import concourse.bass as bass
    import contextlib

    f32 = mybir.dt.float32
    nc = bacc.Bacc(None, target_bir_lowering=False)
    T = {}
    f8 = mybir.dt.float8e4
    T["xkv"] = nc.dram_tensor("xkv", [C, HW], f8, kind="ExternalInput")
    T["xq"] = nc.dram_tensor("xq", [C, HWQ], f8, kind="ExternalInput")
    T["wall"] = nc.dram_tensor("wall", [128, 12 * 512], f8, kind="ExternalInput")
    T["wotp"] = nc.dram_tensor("wotp", [128, 4 * 512], f8, kind="ExternalInput")
    T["colc"] = nc.dram_tensor("colc", [128, 16], f32, kind="ExternalInput")
    T["gadj"] = nc.dram_tensor("gadj", [128, 128], f32, kind="ExternalInput")
    T["outt"] = nc.dram_tensor("outt", [HWQ, C], mybir.dt.float16, kind="ExternalOutput")

    with tile.TileContext(nc) as tc:
        for _ in range(repeat):
            with contextlib.ExitStack() as ctx:
                _emit_body(nc, tc, bass, mybir, ctx, T)
    nc.finalize()
    return nc


def make_in_maps(inputs):
    """Host-side sharding: per-core input dicts."""
    x = np.ascontiguousarray(np.asarray(inputs["x"], dtype=np.float32))
    B = x.shape[0]
    xf = x.reshape(B, C, HW)
    f8 = ml_dtypes.float8_e4m3
    wT8 = {nm: np.asarray(inputs[nm], np.float32).T.astype(f8)
           for nm in ("wq", "wk", "wv")}
    wall = np.empty((128, 12 * 512), f8)
    for i, nm in enumerate(("wk", "wv", "wq")):
        for ci in range(CCH):
            wall[:, i * 2048 + ci * 512:i * 2048 + (ci + 1) * 512] = \
                wT8[nm][ci * 128:(ci + 1) * 128, :]
    woT = np.asarray(inputs["wo"], np.float32).T.astype(f8)
    wotp = np.empty((128, 4 * 512), f8)
    for ci in range(CCH):
        wotp[:, ci * 512:(ci + 1) * 512] = woT[ci * 128:(ci + 1) * 128, :]
    colc = np.empty((128, 16), np.float32)
    for ci in range(CCH):
        sl = slice(ci * 128, (ci + 1) * 128)
        colc[:, 4 * ci + 0] = np.asarray(inputs["gn_w"], np.float32)[sl]
        colc[:, 4 * ci + 1] = np.asarray(inputs["gn_b"], np.float32)[sl]
        colc[:, 4 * ci + 2] = np.asarray(inputs["bq"], np.float32)[sl] * SCALE_H
        colc[:, 4 * ci + 3] = 0.0
    com = {
        "wall": np.ascontiguousarray(wall),
        "wotp": np.ascontiguousarray(wotp),
        "colc": np.ascontiguousarray(colc),
        "gadj": np.ascontiguousarray(
            (np.arange(128)[:, None] // GS == np.arange(128)[None, :] // GS).astype(np.float32)),
    }
    in_maps = []
    for core in range(NCORES):
        b, half = core // 2, core % 2
        m = dict(com)
        m["xkv"] = np.ascontiguousarray(xf[b]).astype(f8)
        m["xq"] = np.ascontiguousarray(xf[b][:, half * HWQ:(half + 1) * HWQ]).astype(f8)
        in_maps.append(m)
    return in_maps


def assemble(inputs, results):
    x = np.asarray(inputs["x"], dtype=np.float32)
    B = x.shape[0]
    xf = x.reshape(B, C, HW)
    bo = np.asarray(inputs["bo"], np.float32)
    bv = np.asarray(inputs["bv"], np.float32)
    wv = np.asarray(inputs["wv"], np.float32)
    wo = np.asarray(inputs["wo"], np.float32)
    gn_w = np.asarray(inputs["gn_w"], np.float32)
    gn_b = np.asarray(inputs["gn_b"], np.float32)
    out = np.empty((B, C, HW), np.float32)
    for core in range(NCORES):
        b, half = core // 2, core % 2
        out[b][:, half * HWQ:(half + 1) * HWQ] = results[core]["outt"].T.astype(np.float32)
    # folded biases: out += Wo@(Wv@b_gn + bv) + bo  (exact f32 GN stats)
    xg = xf.reshape(B, GROUPS, GS * HW)
    mu_g = xg.mean(axis=2)                      # (B, GROUPS)
    var_g = xg.var(axis=2)
    rstd_g = 1.0 / np.sqrt(var_g + EPS)
    a_c = gn_w[None, :] * np.repeat(rstd_g, GS, axis=1)      # (B, C)
    b_c = gn_b[None, :] - np.repeat(mu_g, GS, axis=1) * a_c  # (B, C)
    bve = b_c @ wv.T + bv[None, :]              # (B, C)
    obias = bve @ wo.T + bo[None, :]            # (B, C)
    out += obias[:, :, None]
    out += xf
    return out.reshape(x.shape)


def kernel(**inputs):
    from concourse.bass_utils import run_bass_kernel_spmd
    if "nc" not in _cache:
        _cache["nc"] = build_program(repeat=1)
    nc = _cache["nc"]
    in_maps = make_in_maps(inputs)
    res = run_bass_kernel_spmd(nc, in_maps, list(range(NCORES)))
    return assemble(inputs, res.results)

